# revision 2
# baseline (speedup 1.0000x reference)
"""Causal single-head attention (B=4, S=2048, D=1024) on 8 TRN2 NeuronCores.

V-split variant of the baseline: data-parallel, 2 cores per batch element,
16 query blocks split A/B triangle-balanced as before. NEW: the V projection
is computed HALF per core (own 1024 seq rows, data-driven via xVP input) and
exchanged with the pair partner through a Shared internal DRAM tensor:

  Vh compute (own half) -> 8 chunked DMA writes to S[h] (h = per-core input,
  dynamic register offset) -> WAR-ordered flag -> pairwise AllGather barrier
  (512B) -> cond-gated readback of S[0],S[1] into the absolute-order V tile.

The barrier's ~55us latency hides behind the K/Q projections; AV for the
first slot reads V at ~116us while V is ready by ~95us.

Everything else (KT full, QT, phase-2 scores/exp/transpose/AV) is unchanged.
"""
import ml_dtypes
import numpy as np

import concourse.bacc as bacc
import concourse.bass as bass
import concourse.mybir as mybir
import concourse.tile as tile
from concourse.bass_utils import run_bass_kernel_spmd
from concourse.masks import make_identity

F32 = mybir.dt.float32
BF16 = mybir.dt.bfloat16
U32 = mybir.dt.uint32
AX = mybir.AxisListType
AF = mybir.ActivationFunctionType

P = 128
B, S, D = 4, 2048, 1024
NQ = 1024            # query rows per core
BLOCKS_A = [0, 3, 4, 7, 8, 11, 12, 15]
BLOCKS_B = [1, 2, 5, 6, 9, 10, 13, 14]
NEG = -1e30
SCALE = 1.0 / 32.0   # 1/sqrt(D)

PAIRS = [[0, 1], [2, 3], [4, 5], [6, 7]]
VH_TILE = P * D      # elements per [128,1024] V tile
VH_HALF = 8 * VH_TILE

_CACHE = {}


def _build():
    nc = bacc.Bacc("TRN2", target_bir_lowering=False, debug=False)
    # x chunks pre-paneled host-side: [ck, p, dt, s] = x[ck*512+s, dt*128+p]
    xTP_d = nc.declare_dram_parameter("xTP", [4, P, 8, 512], BF16, isOutput=False)
    xVP_d = nc.declare_dram_parameter("xVP", [2, P, 8, 512], BF16, isOutput=False)
    xqTP_d = nc.declare_dram_parameter("xqTP", [2, P, 8, 512], BF16, isOutput=False)
    wqp_d = nc.declare_dram_parameter("wqp", [8, P, 8, P], BF16, isOutput=False)
    wkp_d = nc.declare_dram_parameter("wkp", [8, P, 8, P], BF16, isOutput=False)
    wvT_d = nc.declare_dram_parameter("wvT", [D, D], BF16, isOutput=False)
    mb_d = nc.declare_dram_parameter("maskb", [P, 8, 256], F32, isOutput=False)
    half_d = nc.declare_dram_parameter("half", [1, 1], U32, isOutput=False)
    out_d = nc.declare_dram_parameter("out", [NQ, D], F32, isOutput=True)

    # shared exchange buffer: [half][kt 8][p 128][e 1024]
    S_x = nc.dram_tensor("vxch", [2, 8, P, D], BF16, kind="Internal",
                         addr_space="Shared")
    gbar = nc.dram_tensor("gbar", [2, P, 2], U32, kind="Internal")

    def wv_tiled(lo, hi):
        return wvT_d.ap().rearrange("(t p) f -> p t f", p=P)[:, :, lo:hi]

    with tile.TileContext(nc) as tc:
        with (
            tc.tile_pool(name="store", bufs=1) as store,
            tc.tile_pool(name="wpool", bufs=2) as wpool,
            tc.tile_pool(name="wqpool", bufs=3) as wqpool,
            tc.tile_pool(name="xpool", bufs=4) as xpool,
            tc.tile_pool(name="small", bufs=4) as smallp,
            tc.tile_pool(name="outp", bufs=3) as outp,
            tc.tile_pool(name="dramp", bufs=1, space="DRAM") as dramp,
            tc.tile_pool(name="ps_a", bufs=1, space="PSUM") as psA,
            tc.tile_pool(name="ps_b", bufs=5, space="PSUM") as psB,
            tc.tile_pool(name="ps_t", bufs=2, space="PSUM") as psT,
        ):
            ident = store.tile([P, P], BF16)
            make_identity(nc, ident[:])
            KT = store.tile([P, 8, S], BF16)    # [e%128, e//128, key]
            V = store.tile([P, 16, D], BF16)    # [k%128, k//128, e]
            QT = store.tile([P, 8, NQ], BF16)   # [e%128, e//128, q]
            Vh = store.tile([P, 8, D], BF16)    # own-half V tiles (local order)

            # h register for the dynamic shared-DRAM write offset
            hreg = nc.sync.alloc_register("hreg")
            nc.sync.reg_load(hreg, half_d[0:1, 0:1])
            hv = nc.sync.snap(hreg, donate=True, min_val=0, max_val=1)

            # ---- DMA emission in first-use order. V half runs first (to
            # launch the exchange barrier as early as possible); its inputs
            # are emitted per-dt interleaved so the first contraction's
            # slices arrive ASAP. KT inputs stream behind. ----
            wv = wpool.tile([P, 8, D], BF16, tag="w")      # [p, dt, e] rhs layout
            xvs = []
            xv = xpool.tile([P, 8, 512], BF16, tag="x")
            nc.sync.dma_start(xv[:, 0:4], xVP_d.ap()[0][:, 0:4])
            nc.sync.dma_start(xv[:, 4:8], xVP_d.ap()[0][:, 4:8])
            xvs.append(xv)
            nc.sync.dma_start(wv[:, :, 0:512], wv_tiled(0, 512))
            nc.sync.dma_start(wv[:, :, 512:1024], wv_tiled(512, 1024))
            xv = xpool.tile([P, 8, 512], BF16, tag="x")
            nc.sync.dma_start(xv[:], xVP_d.ap()[1])
            xvs.append(xv)
            wk = wpool.tile([P, 8, 8, P], BF16, tag="w")   # [p, et, dt, es]
            nc.sync.dma_start(wk[:, 0], wkp_d.ap()[0])
            xts = []
            xt = xpool.tile([P, 8, 512], BF16, tag="x")
            nc.sync.dma_start(xt[:, 0:4], xTP_d.ap()[0][:, 0:4])
            nc.sync.dma_start(xt[:, 4:8], xTP_d.ap()[0][:, 4:8])
            xts.append(xt)
            for et in range(1, 8):
                nc.sync.dma_start(wk[:, et], wkp_d.ap()[et])

            # ---- phase 1a: V half (own keys) + exchange ----
            for i in range(2):
                xt = xvs[i]
                for sub in range(4):
                    for ev in range(2):
                        ps = psB.tile([P, 512], F32, tag="pb")
                        for dt in range(8):
                            nc.tensor.matmul(
                                ps[:],
                                xt[:, dt, sub * P:(sub + 1) * P],
                                wv[:, dt, ev * 512:(ev + 1) * 512],
                                start=(dt == 0),
                                stop=(dt == 7),
                            )
                        nc.any.tensor_copy(
                            Vh[:, i * 4 + sub, ev * 512:(ev + 1) * 512], ps[:]
                        )
                # W: own half tiles -> S[h] (dynamic offset)
                for sub in range(4):
                    t = i * 4 + sub
                    base = S_x.ap()[0][t]       # [P, D] static template
                    dyn = bass.AP(base.tensor, hv * VH_HALF + t * VH_TILE,
                                  base.ap.copy())
                    nc.sync.dma_start(dyn, Vh[:, t, :])

            # flag: WAR hazard against every W tile read => the flag chain
            # cannot start until all 8 shared-DRAM writes completed
            nc.vector.memset(Vh[:, :, 0:2], 1.0)
            Uf = smallp.tile([P, 2], U32, tag="uf")
            nc.vector.tensor_copy(Uf[:], Vh[:, 0, 0:2])
            bflag = dramp.tile([P, 2], U32)
            nc.sync.dma_start(bflag[:], Uf[:])
            nc.gpsimd.collective_compute(
                "AllGather", mybir.AluOpType.bypass, replica_groups=PAIRS,
                ins=[bflag.opt()], outs=[gbar.ap()],
            )
            gsb = smallp.tile([P, 4], U32, tag="gsb")
            for r in range(2):
                nc.sync.dma_start(gsb[:, 2 * r:2 * r + 2], gbar.ap()[r])
            creg = nc.sync.alloc_register("creg")
            nc.sync.reg_load(creg, gsb[0:1, 0:1])
            cv = nc.sync.snap(creg, donate=True, min_val=0, max_val=1)

            # cond-gated readback: S[0] -> V[:,0:8], S[1] -> V[:,8:16]
            for r in range(2):
                for t in range(8):
                    nc.sync.dma_start(
                        V[:, r * 8 + t, :], S_x.ap()[r][t], cond=cv
                    )

            # ---- phase 1b: KT over all 4 absolute chunks ----
            for ck in range(4):
                if ck == 0:
                    xt = xts[0]
                else:
                    xt = xpool.tile([P, 8, 512], BF16, tag="x")
                    nc.sync.dma_start(xt[:], xTP_d.ap()[ck])
                for et in range(8):
                    ps = psB.tile([P, 512], F32, tag="pb")
                    for dt in range(8):
                        nc.tensor.matmul(
                            ps[:],
                            wk[:, et, dt, :],
                            xt[:, dt, :],
                            start=(dt == 0),
                            stop=(dt == 7),
                        )
                    nc.any.tensor_copy(KT[:, et, ck * 512:(ck + 1) * 512], ps[:])

            # ---- phase 1c: QT = wq @ xq^T (et-outer, streamed wq panels) ----
            xqs = []
            for cq in range(2):
                xq = xpool.tile([P, 8, 512], BF16, tag="x")
                nc.sync.dma_start(xq[:], xqTP_d.ap()[cq])
                xqs.append(xq)
            wq_panels = []
            for et in range(8):
                wqp = wqpool.tile([P, 8, P], BF16, tag="wq")
                nc.sync.dma_start(wqp[:], wqp_d.ap()[et])
                wq_panels.append(wqp)
            maskt = store.tile([P, 8, 256], F32)
            nc.sync.dma_start(maskt[:], mb_d.ap())
            for et in range(8):
                wqp = wq_panels[et]
                for cq in range(2):
                    ps = psB.tile([P, 512], F32, tag="pb")
                    for dt in range(8):
                        nc.tensor.matmul(
                            ps[:],
                            wqp[:, dt, :],
                            xqs[cq][:, dt, :],
                            start=(dt == 0),
                            stop=(dt == 7),
                        )
                    nc.any.tensor_copy(QT[:, et, cq * 512:(cq + 1) * 512], ps[:])

            # ---- phase 2: all scores first (largest slot first), then all
            # AVs (smallest first). AV touches V only after every score is
            # done (~t+30us), so the exchange barrier can never stall it. ----
            def emit_av(s, probsT, rec, ltiles):
                for ev in range(2):
                    pav = psB.tile([P, 512], F32, tag="pb")
                    for t in range(ltiles):
                        nc.tensor.matmul(
                            pav[:],
                            probsT[:, t, :],
                            V[:, t, ev * 512:(ev + 1) * 512],
                            start=(t == 0),
                            stop=(t == ltiles - 1),
                        )
                    ot = outp.tile([P, 512], F32, tag="out")
                    nc.scalar.activation(ot[:], pav[:], AF.Copy, scale=rec[:])
                    nc.sync.dma_start(
                        out_d.ap()[s * P:(s + 1) * P, ev * 512:(ev + 1) * 512],
                        ot[:],
                    )

            pending = []
            for s in range(7, -1, -1):
                ltiles = 2 * (s + 1)
                keys = 256 * (s + 1)
                n512 = (s + 1) // 2
                rem = (s + 1) % 2
                nch = n512 + rem
                probs = xpool.tile([P, S], BF16, tag="x")
                probsT = xpool.tile([P, 16, P], BF16, tag="x")
                sums = smallp.tile([P, 8], F32, tag="sums")
                for c in range(nch):
                    is_rem = rem and c == nch - 1
                    w = 256 if is_rem else 512
                    lo = c * 512
                    if is_rem:
                        ps = psA.tile([P, 256], F32, tag="pa")
                    else:
                        ps = psB.tile([P, 512], F32, tag="pb")
                    for dt in range(8):
                        nc.tensor.matmul(
                            ps[:],
                            QT[:, dt, s * P:(s + 1) * P],
                            KT[:, dt, lo:lo + w],
                            start=(dt == 0),
                            stop=(dt == 7),
                        )
                    if lo + w == keys:
                        nc.vector.tensor_add(
                            ps[:, w - 256:w], ps[:, w - 256:w], maskt[:, s, :]
                        )
                    nc.scalar.activation(
                        probs[:, lo:lo + w],
                        ps[:],
                        AF.Exp,
                        scale=SCALE,
                        accum_out=sums[:, c:c + 1],
                    )
                    if s >= 2:
                        nc.scalar.dma_start(
                            probsT[:, lo // P:(lo + w) // P, :],
                            probs[:, lo:lo + w],
                            transpose=True,
                        )
                    else:
                        for t in range(lo // P, (lo + w) // P):
                            pt = psT.tile([P, P], BF16, tag="tr")
                            nc.tensor.transpose(
                                pt[:], probs[:, t * P:(t + 1) * P], ident[:]
                            )
                            nc.any.tensor_copy(probsT[:, t, :], pt[:])
                den = smallp.tile([P, 1], F32, tag="den")
                nc.vector.reduce_sum(den[:], sums[:, :nch], axis=AX.X)
                rec = smallp.tile([P, 1], F32, tag="rec")
                nc.vector.reciprocal(rec[:], den[:])
                pending.append((s, probsT, rec, ltiles))
                if len(pending) > 1:
                    emit_av(*pending.pop(0))
            for p in pending:
                emit_av(*p)
    nc.compile()
    return nc


def _make_masks():
    masks = []
    for blocks in (BLOCKS_A, BLOCKS_B):
        m = np.zeros((P, 8, 256), np.float32)
        for s, j in enumerate(blocks):
            q = j * P + np.arange(P)[:, None]
            k = 256 * s + np.arange(256)[None, :]
            m[:, s, :] = np.where(k <= q, 0.0, NEG)
        masks.append(m)
    return masks


def _bf16(a):
    return np.ascontiguousarray(a.astype(ml_dtypes.bfloat16))


def _panelize(wT):
    return _bf16(wT.reshape(8, P, 8, P).transpose(2, 1, 0, 3))


def _chunk_panels(rows, nck):
    return _bf16(rows.reshape(nck, 512, 8, P).transpose(0, 3, 2, 1))


LAST_RESULT = None


def kernel(x, wq, wk, wv):
    global LAST_RESULT
    x = np.ascontiguousarray(np.asarray(x, dtype=np.float32))
    wq = np.asarray(wq, dtype=np.float32)
    wk = np.asarray(wk, dtype=np.float32)
    wv = np.asarray(wv, dtype=np.float32)

    if "nc" not in _CACHE:
        _CACHE["nc"] = _build()
        _CACHE["masks"] = _make_masks()
    nc = _CACHE["nc"]
    masks = _CACHE["masks"]

    wqp = _panelize(wq.T)
    wkp = _panelize(wk.T)
    wvT = _bf16(wv.T)

    in_maps = []
    for c in range(8):
        b, pat = divmod(c, 2)
        blocks = BLOCKS_A if pat == 0 else BLOCKS_B
        xb = x[b]
        xq = np.concatenate([xb[j * P:(j + 1) * P] for j in blocks], 0)
        xtp = _chunk_panels(xb, 4)
        in_maps.append(
            {
                "xTP": xtp,
                "xVP": np.ascontiguousarray(xtp[2 * pat:2 * pat + 2]),
                "xqTP": _chunk_panels(xq, 2),
                "wqp": wqp,
                "wkp": wkp,
                "wvT": wvT,
                "maskb": masks[pat],
                "half": np.array([[pat]], np.uint32),
            }
        )

    res = run_bass_kernel_spmd(nc, in_maps, core_ids=list(range(8)))
    LAST_RESULT = res

    out = np.empty((B, S, D), np.float32)
    for c in range(8):
        b, pat = divmod(c, 2)
        blocks = BLOCKS_A if pat == 0 else BLOCKS_B
        oc = res.results[c]["out"]
        for si, j in enumerate(blocks):
            out[b, j * P:(j + 1) * P] = oc[si * P:(si + 1) * P]
    return out
